# revision 23
# baseline (speedup 1.0000x reference)
"""Trainium2 Bass kernel for nn_ACAClassifier (soft cellular-automaton update).

Reference computation, per depth d (8 depths):
    mask = sigmoid(state @ W[d].T + b[d])
    t    = 4*l + 2*c + r          (circular neighbors along feature axis)
    nb   = int(t)                 (truncation)
    bits = rule110_table[7 - nb]  == [t>=1] - [t>=4] + [t>=5] - [t>=7]
    state = mask*bits + (1-mask)*state

Sharding: pure data-parallel over the batch axis across 8 NeuronCores.

Per-core structure (depth-outer so all engines pipeline across groups):
  - all 64 batch tiles stay SBUF-resident as [128, G, 514] group tiles
    (columns 0/513 are circular-wrap duplicates so l/c/r are plain slices);
    W.T is streamed from HBM one depth at a time (double-buffered).
  - mask matmul: PE-transpose state chunks -> state_T (stationary), W.T
    chunks (host-pretransposed) as moving operand -> psum, natural layout.
  - neighborhood t: on DVE (exact fp32, matches reference rounding order)
    or on PE via a banded circulant matmul (t_on_pe).
  - bits via fp32 compares on DVE (exact 0/1), fp32 blend.
"""

import sys

for _p in ("/opt/pypackages", "/opt/trn_rl_repo"):
    if _p not in sys.path:
        sys.path.insert(0, _p)

import numpy as np

BATCH = 65536
SIZE = 512
DEPTH = 8
N_CORES = 8
ROWS_PER_CORE = BATCH // N_CORES  # 8192
NTILES = ROWS_PER_CORE // 128     # 64

_NC_CACHE = {}
_BITS_E_OP = None


def get_bits_e_op():
    """Custom fused DVE op: out = ([t>=1]-[t>=4]+[t>=5]-[t>=7]) - c.

    Exact rule-110 bits lookup (integer-edge exact) fused with the
    (bits - state) subtraction: one DVE pass instead of three.
    """
    global _BITS_E_OP
    if _BITS_E_OP is not None:
        return _BITS_E_OP
    from concourse.dve_spec import (AluOp, Bin, One, Spec, Src0, Src1, C0,
                                    C1, Zero)
    from concourse import dve_ops as DO

    # operates on u = t - 4 (the ata that builds t applies bias -4):
    # bits = [u>=-3] - [u>=0] + [u>=1] - [u>=3];  out = bits - c
    body = Bin(AluOp.SUBTRACT,
               Bin(AluOp.ADD,
                   Bin(AluOp.SUBTRACT,
                       Bin(AluOp.IS_GE, Src0, C0),
                       Bin(AluOp.IS_GE, Src0, Zero)),
                   Bin(AluOp.SUBTRACT,
                       Bin(AluOp.IS_GE, Src0, One),
                       Bin(AluOp.IS_GE, Src0, C1))),
               Src1)

    def ref(in0, in1, s0, s1, imm2):
        bits = ((in0 >= s0).astype(np.float32)
                - (in0 >= 0.0).astype(np.float32)
                + (in0 >= 1.0).astype(np.float32)
                - (in0 >= s1).astype(np.float32))
        return (bits - in1).astype(np.float32)

    spec = Spec(body=body, reference=ref)
    name = "RULE_BITS_E_ANT"
    if name not in DO._SUB_OPCODE_FOR_NAME:
        row = max(DO._SUB_OPCODE_FOR_NAME.values()) + 1
        assert row < 0x20, row
        DO._SUB_OPCODE_FOR_NAME[name] = row
    op = DO.DveOp(name, spec, subdim=False, uops_sha={})
    try:
        op.compile("v3")
    except ValueError as e:
        import re
        m = re.search(r"v3: ([0-9a-f]+)", str(e))
        if not m:
            raise
        op = DO.DveOp(name, spec, subdim=False, uops_sha={"v3": m.group(1)})
    op.compile("v3")
    if not any(o.name == name for o in DO.OPS):
        DO.OPS.append(op)
    _BITS_E_OP = op
    return op


def build_nc(ntiles, G=2, with_bias=False, t_on_pe=False, mm_pair=False,
             abs_on_sc=False, c2_on_sc=False, mm_f32r=False,
             num_devices=N_CORES, tmp_bufs=2, mask_bufs=3, st_bufs=4,
             psm_bufs=2, tb_bufs=None, repeat=1, skip_ew=False,
             skip_mm=False):
    """Build + compile the per-core Bass program (depth-outer schedule).

    mm_pair: 3-term bf16-pair mask matmul (s_hi@w_hi + s_hi@w_lo + s_lo@w_hi)
             instead of native fp32 (4 cyc/row -> 3x 1 cyc/row on PE).
    abs_on_sc: compute r1=|t-2.5|, r2=|t-6| on ScalarE; bits via is_lt on DVE.
    c2_on_sc: compute 2*c on ScalarE instead of DVE.
    """
    import concourse.bacc as bacc
    import concourse.mybir as mybir
    import concourse.tile as tile

    assert ntiles % G == 0
    ngroups = ntiles // G
    f32 = mybir.dt.float32
    f32r = mybir.dt.float32r
    bf16 = mybir.dt.bfloat16
    AL = mybir.AluOpType
    AF = mybir.ActivationFunctionType

    nc = bacc.Bacc("TRN2", target_bir_lowering=False, debug=False,
                   num_devices=num_devices)
    rows = ntiles * 128
    x_d = nc.dram_tensor("x", [rows, SIZE], f32, kind="ExternalInput")
    if mm_pair:
        wth_d = nc.dram_tensor("wt_hi", [128, DEPTH, 4, SIZE], bf16,
                               kind="ExternalInput")
        wtl_d = nc.dram_tensor("wt_lo", [128, DEPTH, 4, SIZE], bf16,
                               kind="ExternalInput")
    else:
        wt_d = nc.dram_tensor("wt", [128, DEPTH, 4, SIZE],
                              f32r if mm_f32r else f32,
                              kind="ExternalInput")
    id_d = nc.dram_tensor("ident", [128, 128], f32, kind="ExternalInput")
    if t_on_pe:
        ct_d = nc.dram_tensor("ct", [128, 130], f32, kind="ExternalInput")
    if with_bias:
        ones_d = nc.dram_tensor("ones", [1, 128], f32, kind="ExternalInput")
        b_d = nc.dram_tensor("b", [1, DEPTH * SIZE], f32, kind="ExternalInput")
    out_d = nc.dram_tensor("out", [rows, SIZE], f32, kind="ExternalOutput")

    with tile.TileContext(nc) as tc:
        with (
            tc.tile_pool(name="const", bufs=1) as constp,
            tc.tile_pool(name="state", bufs=1) as statep,
            tc.tile_pool(name="wtp", bufs=2) as wtp,
            tc.tile_pool(name="maskp", bufs=mask_bufs) as maskp,
            tc.tile_pool(name="tmpf", bufs=tmp_bufs) as tmpf,
            tc.tile_pool(name="tmpb", bufs=tmp_bufs) as tmpb,
            tc.tile_pool(name="stp", bufs=st_bufs) as stp,
            tc.tile_pool(name="psA", bufs=2, space="PSUM") as psA,
            tc.tile_pool(name="psM", bufs=psm_bufs, space="PSUM") as psM,
            tc.tile_pool(name="psT2", bufs=2, space="PSUM") as psT2,
        ):
            id_sb = constp.tile([128, 128], f32, tag="id")
            nc.sync.dma_start(id_sb[:], id_d.ap())
            if abs_on_sc:
                biasA = constp.tile([128, 1], f32, tag="biasA")
                nc.vector.memset(biasA[:], -2.5)
                biasB = constp.tile([128, 1], f32, tag="biasB")
                nc.vector.memset(biasB[:], -6.0)
            if t_on_pe:
                ct_sb = constp.tile([128, 130], f32, tag="ct")
                nc.sync.dma_start(ct_sb[:], ct_d.ap())
            if with_bias:
                ones_sb = constp.tile([1, 128], f32, tag="ones")
                nc.sync.dma_start(ones_sb[:], ones_d.ap())
                b_sb = constp.tile([1, DEPTH * SIZE], f32, tag="b")
                nc.sync.dma_start(b_sb[:], b_d.ap())

            x_ap = x_d.ap()
            wt_ap = None if mm_pair else wt_d.ap()
            out_ap = out_d.ap()

            sts = [statep.tile([128, G, SIZE + 2], f32, tag=f"st{g}",
                               name=f"st{g}")
                   for g in range(ngroups)]

            for rep in range(repeat):
                for g in range(ngroups):
                    st = sts[g]
                    for i in range(G):
                        r0 = (g * G + i) * 128
                        nc.sync.dma_start(st[:, i, 1:SIZE + 1],
                                          x_ap[r0:r0 + 128, :])
                    nc.vector.tensor_copy(st[:, :, 0:1], st[:, :, SIZE:SIZE + 1])
                    nc.vector.tensor_copy(st[:, :, SIZE + 1:SIZE + 2],
                                          st[:, :, 1:2])
                if skip_mm:
                    mask_const = maskp.tile([128, G, SIZE], f32, tag="maskc")
                    nc.vector.memset(mask_const[:], 0.5)

                for d in range(DEPTH):
                    if not skip_mm:
                        if mm_pair:
                            wth_sb = wtp.tile([128, 4, SIZE], bf16, tag="wth")
                            nc.sync.dma_start(wth_sb[:], wth_d.ap()[:, d, :, :])
                            wtl_sb = wtp.tile([128, 4, SIZE], bf16, tag="wtl")
                            nc.sync.dma_start(wtl_sb[:], wtl_d.ap()[:, d, :, :])
                        else:
                            wt_sb = wtp.tile([128, 4, SIZE],
                                             f32r if mm_f32r else f32,
                                             tag="wt")
                            nc.sync.dma_start(wt_sb[:], wt_ap[:, d, :, :])
                    for g in range(ngroups):
                        st = sts[g]
                        if skip_mm:
                            mask = mask_const
                        else:
                            pM = psM.tile([128, G, SIZE], f32, tag="pM")
                            mask = maskp.tile([128, G, SIZE], f32, tag="mask")
                        if t_on_pe:
                            g4 = tmpb.tile([128, G, SIZE], bf16, tag="g4")
                            g7 = tmpb.tile([128, G, SIZE], bf16, tag="g7")
                        for i in range(G if not skip_mm else 0):
                            pT = psA.tile([128, SIZE], f32, tag="pT")
                            for j in range(4):
                                nc.tensor.transpose(
                                    pT[:, j * 128:(j + 1) * 128],
                                    st[:, i, 1 + j * 128:1 + (j + 1) * 128],
                                    id_sb[:],
                                )
                            if mm_pair:
                                sTh = stp.tile([128, SIZE], bf16, tag="sTh")
                                nc.scalar.copy(sTh[:], pT[:])
                                sTl = stp.tile([128, SIZE], bf16, tag="sTl")
                                nc.vector.tensor_tensor(sTl[:], pT[:], sTh[:],
                                                        AL.subtract)
                                for j in range(4):
                                    cj = slice(j * 128, (j + 1) * 128)
                                    nc.tensor.matmul(
                                        pM[:, i, :], sTh[:, cj],
                                        wth_sb[:, j, :],
                                        start=(j == 0), stop=False)
                                    nc.tensor.matmul(
                                        pM[:, i, :], sTh[:, cj],
                                        wtl_sb[:, j, :],
                                        start=False, stop=False)
                                    nc.tensor.matmul(
                                        pM[:, i, :], sTl[:, cj],
                                        wth_sb[:, j, :],
                                        start=False,
                                        stop=(j == 3 and not with_bias))
                            else:
                                sT = stp.tile([128, SIZE],
                                              f32r if mm_f32r else f32,
                                              tag="sT")
                                nc.scalar.copy(sT[:], pT[:])
                                for j in range(4):
                                    nc.tensor.matmul(
                                        pM[:, i, :],
                                        sT[:, j * 128:(j + 1) * 128],
                                        wt_sb[:, j, :],
                                        start=(j == 0),
                                        stop=(j == 3 and not with_bias),
                                    )
                            if with_bias:
                                nc.tensor.matmul(
                                    pM[:, i, :],
                                    ones_sb[0:1, :],
                                    b_sb[0:1, d * SIZE:(d + 1) * SIZE],
                                    start=False,
                                    stop=True,
                                )
                            if t_on_pe:
                                # banded circulant t = 4l + 2c + r on PE:
                                # chunk j owns out cols [128j, 128j+127];
                                # cross-chunk neighbor contributions are
                                # added by 1-col accumulate fixups.
                                pT2 = psT2.tile([128, SIZE], f32, tag="pT2")
                                for j in range(4):
                                    nc.tensor.matmul(
                                        pT2[:, j * 128:(j + 1) * 128],
                                        sT[:, j * 128:(j + 1) * 128],
                                        ct_sb[:, 1:129],
                                        start=(j == 0), stop=False,
                                        skip_group_check=True)
                                for j in range(4):
                                    ca = (j * 128 + 128) % SIZE
                                    nc.tensor.matmul(
                                        pT2[:, ca:ca + 1],
                                        sT[:, j * 128:(j + 1) * 128],
                                        ct_sb[:, 129:130],
                                        start=False, stop=False,
                                        skip_group_check=True)
                                    cb = (j * 128 + SIZE - 1) % SIZE
                                    nc.tensor.matmul(
                                        pT2[:, cb:cb + 1],
                                        sT[:, j * 128:(j + 1) * 128],
                                        ct_sb[:, 0:1],
                                        start=False, stop=(j == 3),
                                        skip_group_check=True)
                                # per-tile compares straight from PSUM
                                nc.vector.tensor_scalar(
                                    g4[:, i, :], pT2[:], 4.0, None, AL.is_ge)
                                nc.vector.tensor_scalar(
                                    g7[:, i, :], pT2[:], 7.0, None, AL.is_ge)
                                nc.vector.scalar_tensor_tensor(
                                    g4[:, i, :], pT2[:], 1.0, g4[:, i, :],
                                    AL.is_ge, AL.subtract)
                                nc.vector.scalar_tensor_tensor(
                                    g7[:, i, :], pT2[:], 5.0, g7[:, i, :],
                                    AL.is_ge, AL.subtract)
                        if not skip_mm:
                            nc.scalar.activation(mask[:], pM[:], AF.Sigmoid)
                        if skip_ew:
                            continue

                        l_ap = st[:, :, 0:SIZE]
                        c_ap = st[:, :, 1:SIZE + 1]
                        r_ap = st[:, :, 2:SIZE + 2]
                        tb = tmpf.tile([128, G, SIZE], f32, tag="tb",
                                       bufs=tb_bufs)
                        if not t_on_pe:
                            c2 = tmpf.tile([128, G, SIZE], f32, tag="c2")
                            # u = 4*l + 2*c (both scalings exact, one add)
                            if c2_on_sc:
                                nc.scalar.mul(c2[:], c_ap, 2.0)
                            else:
                                nc.vector.tensor_scalar(c2[:], c_ap, 2.0, None,
                                                        AL.mult)
                            nc.vector.scalar_tensor_tensor(
                                c2[:], l_ap, 4.0, c2[:], AL.mult, AL.add)
                            # t = u + r
                            nc.vector.tensor_tensor(tb[:], c2[:], r_ap, AL.add)
                            t_src = tb[:]
                            g4 = tmpb.tile([128, G, SIZE], bf16, tag="g4")
                            g7 = tmpb.tile([128, G, SIZE], bf16, tag="g7")
                            if abs_on_sc:
                                # bits=1 iff t in [1,4) u [5,7):
                                # r1=|t-2.5|<1.5, r2=|t-6|<1 (exact-integer
                                # edge cases are measure-zero and accepted)
                                ra = tmpf.tile([128, G, SIZE], f32, tag="ra")
                                rb = tmpf.tile([128, G, SIZE], f32, tag="rb")
                                nc.scalar.activation(ra[:], t_src, AF.Abs,
                                                     bias=biasA[:])
                                nc.scalar.activation(rb[:], t_src, AF.Abs,
                                                     bias=biasB[:])
                                nc.vector.tensor_scalar(g4[:], ra[:], 1.5,
                                                        None, AL.is_lt)
                                nc.vector.tensor_scalar(g7[:], rb[:], 1.0,
                                                        None, AL.is_lt)
                            else:
                                nc.vector.tensor_scalar(g4[:], t_src, 4.0,
                                                        None, AL.is_ge)
                                nc.vector.tensor_scalar(g7[:], t_src, 7.0,
                                                        None, AL.is_ge)
                                # d1 = [t>=1]-[t>=4]; d2 = [t>=5]-[t>=7]
                                nc.vector.scalar_tensor_tensor(
                                    g4[:], t_src, 1.0, g4[:], AL.is_ge,
                                    AL.subtract)
                                nc.vector.scalar_tensor_tensor(
                                    g7[:], t_src, 5.0, g7[:], AL.is_ge,
                                    AL.subtract)
                        # bits = d1 + d2  (exact 0/1 in bf16)
                        nc.vector.tensor_tensor(g4[:], g4[:], g7[:], AL.add)
                        # e = bits - state
                        nc.vector.scalar_tensor_tensor(
                            tb[:], c_ap, -1.0, g4[:], AL.mult, AL.add)
                        # f = mask * e
                        nc.vector.tensor_tensor(tb[:], mask[:], tb[:], AL.mult)
                        # state += f   (in-place)
                        nc.vector.tensor_tensor(st[:, :, 1:SIZE + 1], tb[:],
                                                c_ap, AL.add)
                        # refresh circular-wrap pad columns
                        nc.vector.tensor_copy(st[:, :, 0:1],
                                              st[:, :, SIZE:SIZE + 1])
                        nc.vector.tensor_copy(st[:, :, SIZE + 1:SIZE + 2],
                                              st[:, :, 1:2])

                for g in range(ngroups):
                    st = sts[g]
                    for i in range(G):
                        r0 = (g * G + i) * 128
                        nc.sync.dma_start(out_ap[r0:r0 + 128, :],
                                          st[:, i, 1:SIZE + 1])

    nc.compile()
    return nc


def _host_inputs(W, b, with_bias, t_on_pe, mm_pair=False):
    import ml_dtypes

    W = np.asarray(W, dtype=np.float32)
    # wt[p, d, j, n] = W[d][n, j*128+p]
    wt = np.ascontiguousarray(
        W.transpose(0, 2, 1).reshape(DEPTH, 4, 128, SIZE).transpose(2, 0, 1, 3))
    common = {
        "ident": np.eye(128, dtype=np.float32),
    }
    if mm_pair:
        wt_hi = wt.astype(ml_dtypes.bfloat16)
        wt_lo = (wt - wt_hi.astype(np.float32)).astype(ml_dtypes.bfloat16)
        common["wt_hi"] = wt_hi
        common["wt_lo"] = wt_lo
    else:
        common["wt"] = wt
    if t_on_pe:
        # ct[k, m]: coeff of s_k in window col m: t_n = 4 s_{n-1} + 2 s_n
        # + s_{n+1}; main matmuls use ct[:,1:129], fixups cols 0 and 129.
        ct = np.zeros((128, 130), dtype=np.float32)
        for k in range(128):
            ct[k, k] = 1.0
            ct[k, k + 1] = 2.0
            ct[k, k + 2] = 4.0
        common["ct"] = ct
    if with_bias:
        common["ones"] = np.ones((1, 128), dtype=np.float32)
        common["b"] = np.ascontiguousarray(
            np.asarray(b, dtype=np.float32).reshape(1, DEPTH * SIZE))
    return common


def build_nc_v2(ntiles, G=4, with_bias=False, repeat=1, num_devices=N_CORES,
                wt_bufs=1, nat_bufs=1, st_bufs=2, mask_bufs=2, tmp_bufs=2,
                g_bufs=1, psm_bufs=2, r_halves=2, feed_span=1,
                eng_sh="scalar", eng_sl="gpsimd", eng_r="scalar",
                eng_g="vector", eng_bits="vector", eng_e="vector",
                eng_f="vector",
                eng_sp="gpsimd", use_ata=True, v2=True,
                skip_ew=False, skip_mm=False):
    """v2: no PE transposes (XBAR DMA transpose of the bf16 pair),
    3-term bf16-pair matmul, fused DVE ops, per-op engine assignment.

    Per depth, per group of G tiles (state resident as [128, G, 514] f32):
      split:  sh = bf16(state)         [eng_sh]
              sl = state - sh  (bf16)  [eng_sl]
      dmaT:   sTh, sTl = xbar-transpose(sh, sl)   [SP hwdge, 2 instrs]
      mm:     12 matmuls per tile -> pM psum      [PE]
      mask:   sigmoid(pM) -> sbuf                 [Act]
      t:      t = (4l + r) + 2c  (2x affine_then_add)   [DVE]
      bits:   r1=|t-2.5|, r2=|t-6| (fused ts absmax, 2x mode)
              g4=r1<1.5, g7=r2<1 (bf16), bits=g4+g7
      blend:  e = bits - c; f = mask*e; state += f  [DVE/gpsimd]
    """
    import concourse.bacc as bacc
    import concourse.mybir as mybir
    import concourse.tile as tile

    assert not with_bias, "v2 requires zero bias (falls back to v1)"
    assert ntiles % G == 0
    ngroups = ntiles // G
    f32 = mybir.dt.float32
    bf16 = mybir.dt.bfloat16
    AL = mybir.AluOpType
    AF = mybir.ActivationFunctionType

    nc = bacc.Bacc("TRN2", target_bir_lowering=False, debug=False,
                   num_devices=num_devices)
    rows = ntiles * 128
    x_d = nc.dram_tensor("x", [rows, SIZE], f32, kind="ExternalInput")
    wth_d = nc.dram_tensor("wt_hi", [128, DEPTH, 4, SIZE], bf16,
                           kind="ExternalInput")
    wtl_d = nc.dram_tensor("wt_lo", [128, DEPTH, 4, SIZE], bf16,
                           kind="ExternalInput")
    out_d = nc.dram_tensor("out", [rows, SIZE], f32, kind="ExternalOutput")

    def eng(name):
        return {"scalar": nc.scalar, "vector": nc.vector,
                "gpsimd": nc.gpsimd}[name]

    with tile.TileContext(nc) as tc:
        with (
            tc.tile_pool(name="const", bufs=1) as constp,
            tc.tile_pool(name="state", bufs=1) as statep,
            tc.tile_pool(name="wtp", bufs=wt_bufs) as wtp,
            tc.tile_pool(name="natp", bufs=nat_bufs) as natp,
            tc.tile_pool(name="stp", bufs=st_bufs) as stp,
            tc.tile_pool(name="maskp", bufs=mask_bufs) as maskp,
            tc.tile_pool(name="tmpp", bufs=tmp_bufs) as tmpp,
            tc.tile_pool(name="gp", bufs=g_bufs) as gp,
            tc.tile_pool(name="psM", bufs=psm_bufs, space="PSUM") as psM,
        ):
            x_ap = x_d.ap()
            out_ap = out_d.ap()
            if eng_r == "scalar":
                biasA = constp.tile([128, 1], f32, tag="biasA")
                nc.vector.memset(biasA[:], -2.5)
                biasB = constp.tile([128, 1], f32, tag="biasB")
                nc.vector.memset(biasB[:], -6.0)

            sts = [statep.tile([128, G, SIZE + 2], f32, tag=f"st{g}",
                               name=f"st{g}")
                   for g in range(ngroups)]

            for rep in range(repeat):
                for g in range(ngroups):
                    st = sts[g]
                    for i in range(G):
                        r0 = (g * G + i) * 128
                        nc.sync.dma_start(st[:, i, 1:SIZE + 1],
                                          x_ap[r0:r0 + 128, :])
                    nc.vector.tensor_copy(st[:, :, 0:1],
                                          st[:, :, SIZE:SIZE + 1])
                    nc.vector.tensor_copy(st[:, :, SIZE + 1:SIZE + 2],
                                          st[:, :, 1:2])
                if skip_mm:
                    mask_const = maskp.tile([128, G, SIZE], f32, tag="maskc")
                    nc.vector.memset(mask_const[:], 0.5)

                for d in range(DEPTH):
                    if not skip_mm:
                        wth_sb = wtp.tile([128, 4, SIZE], bf16, tag="wth")
                        nc.sync.dma_start(wth_sb[:], wth_d.ap()[:, d, :, :])
                        wtl_sb = wtp.tile([128, 4, SIZE], bf16, tag="wtl")
                        nc.sync.dma_start(wtl_sb[:], wtl_d.ap()[:, d, :, :])
                    for g in range(ngroups):
                        st = sts[g]
                        l_ap = st[:, :, 0:SIZE]
                        c_ap = st[:, :, 1:SIZE + 1]
                        r_ap = st[:, :, 2:SIZE + 2]

                        if skip_mm:
                            mask = mask_const
                        else:
                            # split state into exact bf16 pair, then XBAR
                            # transpose both halves; feeds batched over
                            # feed_span consecutive groups per DMA.
                            gi = g % feed_span
                            if gi == 0:
                                sh_sp = natp.tile([128, feed_span * G, SIZE],
                                                  bf16, tag="sh")
                                sl_sp = natp.tile([128, feed_span * G, SIZE],
                                                  bf16, tag="sl")
                                for g2 in range(g, g + feed_span):
                                    i2 = (g2 - g) * G
                                    c2_ap = sts[g2][:, :, 1:SIZE + 1]
                                    shs = sh_sp[:, i2:i2 + G, :]
                                    if eng_sh == "scalar":
                                        nc.scalar.copy(shs, c2_ap)
                                    else:
                                        eng(eng_sh).tensor_copy(shs, c2_ap)
                                    eng(eng_sl).tensor_tensor(
                                        sl_sp[:, i2:i2 + G, :], c2_ap, shs,
                                        AL.subtract)
                                sTh = stp.tile([128, feed_span * G * 4, 128],
                                               bf16, tag="sTh")
                                nc.sync.dma_start_transpose(sTh[:], sh_sp[:])
                                sTl = stp.tile([128, feed_span * G * 4, 128],
                                               bf16, tag="sTl")
                                nc.sync.dma_start_transpose(sTl[:], sl_sp[:])
                                span_sT = (sTh, sTl)
                            sTh, sTl = span_sT

                            pM = psM.tile([128, G, SIZE], f32, tag="pM")
                            for i in range(G):
                                for j in range(4):
                                    k = (gi * G + i) * 4 + j
                                    nc.tensor.matmul(
                                        pM[:, i, :], sTh[:, k, :],
                                        wth_sb[:, j, :],
                                        start=(j == 0), stop=False)
                                    nc.tensor.matmul(
                                        pM[:, i, :], sTh[:, k, :],
                                        wtl_sb[:, j, :],
                                        start=False, stop=False)
                                    nc.tensor.matmul(
                                        pM[:, i, :], sTl[:, k, :],
                                        wth_sb[:, j, :],
                                        start=False, stop=(j == 3))
                            mask = maskp.tile([128, G, SIZE], f32, tag="mask")
                            nc.scalar.activation(mask[:], pM[:], AF.Sigmoid)
                        if skip_ew:
                            continue

                        # t = 4l + 2c + r (exact fp32)
                        t = tmpp.tile([128, G, SIZE], f32, tag="tmp")
                        if use_ata:
                            nc.vector.affine_then_add(t[:], l_ap, r_ap,
                                                      4.0, 0.0)
                            nc.vector.affine_then_add(t[:], c_ap, t[:],
                                                      2.0, 0.0)
                        else:
                            nc.vector.tensor_scalar(t[:], l_ap, 4.0, None,
                                                    AL.mult)
                            nc.vector.scalar_tensor_tensor(
                                t[:], c_ap, 2.0, t[:], AL.mult, AL.add)
                            nc.vector.tensor_tensor(t[:], t[:], r_ap, AL.add)
                        # bits: r1=|t-2.5|<1.5 (covers [1,4)), r2=|t-6|<1
                        # (covers [5,7)); exact-integer edges measure-zero.
                        # r1 staged through a half-group scratch to save SBUF;
                        # r2 computed in place over t.
                        g4 = gp.tile([128, G, SIZE], bf16, tag="g4")
                        gh = G // r_halves
                        for h in range(r_halves):
                            hs = slice(h * gh, (h + 1) * gh)
                            r1 = tmpp.tile([128, gh, SIZE], f32, tag="rh",
                                           bufs=1)
                            if eng_r == "scalar":
                                nc.scalar.activation(r1[:], t[:, hs, :],
                                                     AF.Abs, bias=biasA[:])
                            else:
                                nc.vector.tensor_scalar(r1[:], t[:, hs, :],
                                                        2.5, None, AL.subtract)
                                nc.vector.tensor_scalar(r1[:], r1[:], 0.0,
                                                        None, AL.abs_max)
                            nc.vector.tensor_scalar(g4[:, hs, :], r1[:],
                                                    1.5, None, AL.is_lt)
                        if eng_r == "scalar":
                            nc.scalar.activation(t[:], t[:], AF.Abs,
                                                 bias=biasB[:])
                        else:
                            nc.vector.tensor_scalar(t[:], t[:], 6.0, None,
                                                    AL.subtract)
                            nc.vector.tensor_scalar(t[:], t[:], 0.0, None,
                                                    AL.abs_max)
                        g7 = gp.tile([128, G, SIZE], bf16, tag="g7")
                        eng(eng_g).tensor_scalar(g7[:], t[:], 1.0, None,
                                                 AL.is_lt)
                        eng(eng_bits).tensor_tensor(g4[:], g4[:], g7[:],
                                                    AL.add)
                        # blend: e = bits - c; f = mask*e; state += f
                        tb = tmpp.tile([128, G, SIZE], f32, tag="tmp")
                        eng(eng_e).scalar_tensor_tensor(
                            tb[:], c_ap, -1.0, g4[:], AL.mult, AL.add)
                        eng(eng_f).tensor_tensor(tb[:], mask[:], tb[:],
                                                 AL.mult)
                        eng(eng_sp).tensor_tensor(c_ap, tb[:], c_ap, AL.add)
                        # refresh circular-wrap pad columns
                        nc.vector.tensor_copy(st[:, :, 0:1],
                                              st[:, :, SIZE:SIZE + 1])
                        nc.vector.tensor_copy(st[:, :, SIZE + 1:SIZE + 2],
                                              st[:, :, 1:2])

                for g in range(ngroups):
                    st = sts[g]
                    for i in range(G):
                        r0 = (g * G + i) * 128
                        nc.sync.dma_start(out_ap[r0:r0 + 128, :],
                                          st[:, i, 1:SIZE + 1])

    nc.compile()
    return nc


def build_nc_v4(ntiles, CH=4, NI=2, repeat=1, num_devices=N_CORES,
                st_bufs=3, nat_bufs=2, stp_bufs=2, mask_bufs=2, t_bufs=2,
                g_bufs=2, e_bufs=2, psm_bufs=2,
                eng_sl="gpsimd", eng_t1="vector", eng_t2="vector",
                eng_g="vector", eng_b="vector", eng_e="vector",
                eng_f="vector", eng_u="vector", eng_pad="vector",
                feed_lead=2, blend_lag=1, trans_q="split", io_q="sp",
                stt_forms=False, bits_mod=False, with_bias=False,
                skip_ew=False, skip_mm=False):
    """v4: tile-outer schedule. Each chunk of CH row-tiles runs through all
    8 depths; NI chunks are interleaved so PE never idles. W (bf16 pair,
    host-pretransposed) stays SBUF-resident for all depths. State transposes
    via XBAR DMA of the bf16 split pair. Bits via the mod trick:
      bits' = [t>=7] - [(t mod 4)>=1]  ( == -bits )
      e' = c + bits';  f' = mask*e';  c -= f'
    Engine assignment per op is configurable; "alt" entries alternate
    vector/gpsimd per (chunk,depth) parity for fractional balance.
    """
    import concourse.bacc as bacc
    import concourse.mybir as mybir
    import concourse.tile as tile

    assert not with_bias, "v4 requires zero bias (falls back to v1)"
    assert ntiles % CH == 0
    nchunks = ntiles // CH
    f32 = mybir.dt.float32
    bf16 = mybir.dt.bfloat16
    AL = mybir.AluOpType
    AF = mybir.ActivationFunctionType

    bits_e_op = get_bits_e_op()
    nc = bacc.Bacc("TRN2", target_bir_lowering=False, debug=False,
                   num_devices=num_devices)
    rows = ntiles * 128
    x_d = nc.dram_tensor("x", [rows, SIZE], f32, kind="ExternalInput")
    wth_d = nc.dram_tensor("wt_hi", [128, DEPTH, 4, SIZE], bf16,
                           kind="ExternalInput")
    wtl_d = nc.dram_tensor("wt_lo", [128, DEPTH, 4, SIZE], bf16,
                           kind="ExternalInput")
    out_d = nc.dram_tensor("out", [rows, SIZE], f32, kind="ExternalOutput")

    def eng(name, k):
        if isinstance(name, (tuple, list)):
            name = name[k % len(name)]
        return {"vector": nc.vector, "gpsimd": nc.gpsimd,
                "scalar": nc.scalar}[name]

    def tt(e_, out, a, b_, op, k):
        # Pool only supports plain TensorTensor/TensorScalar/TensorCopy
        eng(e_, k).tensor_tensor(out, a, b_, op)

    with tile.TileContext(nc) as tc:
        with (
            tc.tile_pool(name="wres", bufs=1) as wres,
            tc.tile_pool(name="statep", bufs=st_bufs) as statep,
            tc.tile_pool(name="natp", bufs=nat_bufs) as natp,
            tc.tile_pool(name="stp", bufs=stp_bufs) as stp,
            tc.tile_pool(name="maskp", bufs=mask_bufs) as maskp,
            tc.tile_pool(name="tp", bufs=t_bufs) as tp,
            tc.tile_pool(name="gp", bufs=g_bufs) as gp,
            tc.tile_pool(name="ep", bufs=e_bufs) as ep,
            tc.tile_pool(name="psM", bufs=psm_bufs, space="PSUM") as psM,
        ):
            x_ap = x_d.ap()
            out_ap = out_d.ap()

            wth_sb = wres.tile([128, DEPTH, 4, SIZE], bf16, tag="wth")
            wtl_sb = wres.tile([128, DEPTH, 4, SIZE], bf16, tag="wtl")

            assert nchunks % NI == 0
            for rep in range(repeat):
                qio = {"sp": nc.sync, "act": nc.scalar,
                       "gpsimd": nc.gpsimd}[io_q]
                next_sts = {}

                def load_chunk(w, j):
                    ci = w * NI + j
                    st = statep.tile([128, CH, SIZE + 2], f32, tag="st")
                    for i in range(CH):
                        r0 = (ci * CH + i) * 128
                        qio.dma_start(st[:, i, 1:SIZE + 1],
                                      x_ap[r0:r0 + 128, :])
                    nc.vector.tensor_copy(st[:, :, 0:1],
                                          st[:, :, SIZE:SIZE + 1])
                    nc.vector.tensor_copy(st[:, :, SIZE + 1:SIZE + 2],
                                          st[:, :, 1:2])
                    return st

                for w in range(nchunks // NI):
                    if rep == 0 and w == 0:
                        # depth-0 weights first so the first matmuls are fed;
                        # later depths stream just-in-time during wave 0
                        nc.sync.dma_start(wth_sb[:, 0, :, :],
                                          wth_d.ap()[:, 0, :, :])
                        nc.sync.dma_start(wtl_sb[:, 0, :, :],
                                          wtl_d.ap()[:, 0, :, :])
                    sts = [next_sts.pop(j, None) for j in range(NI)]
                    for j in range(NI):
                        if sts[j] is None:
                            sts[j] = load_chunk(w, j)

                    # flat software pipeline over steps (d, j): the feed
                    # (cast + XBAR transposes) leads by feed_lead steps,
                    # crossing depth boundaries; the blend trails by one.
                    nsteps = DEPTH * NI
                    feeds = {}
                    pend = []

                    def emit_feed(srec):
                        d, j = srec
                        st = sts[j]
                        c_ap = st[:, :, 1:SIZE + 1]
                        k = (w * NI + j) * DEPTH + d
                        sh = natp.tile([128, CH, SIZE], bf16, tag="sh")
                        nc.scalar.copy(sh[:], c_ap)
                        sl = natp.tile([128, CH, SIZE], bf16, tag="sl")
                        tt(eng_sl, sl[:], c_ap, sh[:], AL.subtract, k)
                        qh = nc.scalar if trans_q in ("act", "split") \
                            else nc.sync
                        ql = nc.scalar if trans_q == "act" else nc.sync
                        sTh = stp.tile([128, CH * 4, 128], bf16, tag="sTh")
                        qh.dma_start_transpose(sTh[:], sh[:])
                        sTl = stp.tile([128, CH * 4, 128], bf16, tag="sTl")
                        ql.dma_start_transpose(sTl[:], sl[:])
                        feeds[(d, j)] = (sTh, sTl)

                    def emit_blend():
                        st, mask, e, k, d, j = pend.pop(0)
                        c_ap = st[:, :, 1:SIZE + 1]
                        tt(eng_f, e[:], mask[:], e[:], AL.mult, k)
                        tt(eng_u, c_ap, c_ap, e[:], AL.add, k)
                        if d == DEPTH - 1:
                            ci = w * NI + j
                            for i in range(CH):
                                r0 = (ci * CH + i) * 128
                                qio.dma_start(out_ap[r0:r0 + 128, :],
                                              st[:, i, 1:SIZE + 1])
                            if w + 1 < nchunks // NI:
                                # prefetch the next wave's chunk j now; the
                                # tile ring serializes on the out-store read
                                next_sts[j] = load_chunk(w + 1, j)

                    L = min(feed_lead, nsteps)
                    # blend(s) is emitted at the END of step s+blend_lag; the
                    # feed for step s+NI (same chunk, next depth) is emitted
                    # at the START of step s+NI-L and reads the state that
                    # blend(s)'s update writes -- so require
                    # blend_lag <= NI - L - 1.
                    assert blend_lag <= NI - L - 1, (NI, L, blend_lag)
                    for sidx in range(-L if not skip_mm else 0, nsteps):
                        if not skip_mm and sidx + L < nsteps:
                            emit_feed(divmod(sidx + L, NI))
                        if sidx < 0:
                            continue
                        d, j = divmod(sidx, NI)
                        if rep == 0 and w == 0 and j == 0 and d < DEPTH - 1:
                            nc.sync.dma_start(wth_sb[:, d + 1, :, :],
                                              wth_d.ap()[:, d + 1, :, :])
                            nc.sync.dma_start(wtl_sb[:, d + 1, :, :],
                                              wtl_d.ap()[:, d + 1, :, :])
                        ci = w * NI + j
                        st = sts[j]
                        k = ci * DEPTH + d
                        l_ap = st[:, :, 0:SIZE]
                        c_ap = st[:, :, 1:SIZE + 1]
                        r_ap = st[:, :, 2:SIZE + 2]

                        if not skip_mm:
                            sTh, sTl = feeds.pop((d, j))
                            pM = psM.tile([128, CH, SIZE], f32, tag="pM")
                            for i in range(CH):
                                # hi-pair matmuls first; sTl only needed for
                                # the trailing lo@hi group
                                for jj in range(4):
                                    kk = i * 4 + jj
                                    nc.tensor.matmul(
                                        pM[:, i, :], sTh[:, kk, :],
                                        wth_sb[:, d, jj, :],
                                        start=(jj == 0), stop=False,
                                        skip_group_check=True)
                                    nc.tensor.matmul(
                                        pM[:, i, :], sTh[:, kk, :],
                                        wtl_sb[:, d, jj, :],
                                        start=False, stop=False,
                                        skip_group_check=True)
                                for jj in range(4):
                                    kk = i * 4 + jj
                                    nc.tensor.matmul(
                                        pM[:, i, :], sTl[:, kk, :],
                                        wth_sb[:, d, jj, :],
                                        start=False, stop=(jj == 3),
                                        skip_group_check=True)
                            mask = maskp.tile([128, CH, SIZE], f32,
                                              tag="mask")
                            nc.scalar.activation(mask[:], pM[:], AF.Sigmoid)
                        else:
                            mask = maskp.tile([128, CH, SIZE], f32,
                                              tag="mask")
                            nc.vector.memset(mask[:], 0.5)
                        if skip_ew:
                            continue

                        # refresh circular pads from the previous depth's
                        # update (same dependency as t1: no extra blocking);
                        # the final depth needs no pads (output is 1..512)
                        if d > 0:
                            en = eng(eng_pad, k)
                            en.tensor_copy(st[:, :, 0:1],
                                           st[:, :, SIZE:SIZE + 1])
                            en.tensor_copy(st[:, :, SIZE + 1:SIZE + 2],
                                           st[:, :, 1:2])
                        # t = (4l + r) + (2c - 4), exact fp32 (shifted by -4
                        # so the bits op thresholds are {-3, 0, 1, 3})
                        t = tp.tile([128, CH, SIZE], f32, tag="t")
                        eng(eng_t1, k).affine_then_add(t[:], l_ap, r_ap,
                                                       4.0, 0.0)
                        eng(eng_t2, k).affine_then_add(t[:], c_ap, t[:],
                                                       2.0, -4.0)
                        # e = bits(t) - c via the fused custom DVE op
                        e = ep.tile([128, CH, SIZE], f32, tag="e")
                        nc.vector._custom_dve(bits_e_op, out=e[:], in0=t[:],
                                              in1=c_ap, s0=-3.0, s1=3.0)
                        pend.append((st, mask, e, k, d, j))
                        if len(pend) > blend_lag:
                            emit_blend()
                    while pend:
                        emit_blend()

    nc.compile()
    return nc


# default configuration used by kernel(): depth-outer schedule, 3-term
# bf16-pair mask matmul, |t-c| bits layer on ScalarE, 2*c on ScalarE,
# triple-buffered mask psum.
CFG_V1 = dict(G=2, t_on_pe=False, mm_pair=True, abs_on_sc=True, c2_on_sc=True,
              psm_bufs=3)
CFG = dict(v4=True, CH=2, NI=8, st_bufs=11, stp_bufs=3, nat_bufs=3,
           psm_bufs=4, t_bufs=1, e_bufs=4, mask_bufs=4, trans_q="sp",
           io_q="sp", eng_f=("vector", "gpsimd"),
           eng_u=("gpsimd", "vector"), eng_pad="gpsimd", blend_lag=2)


def build_any(ntiles, with_bias=False, cfg=None, **kw):
    cfg = dict(CFG if cfg is None else cfg)
    cfg.update(kw)
    if cfg.pop("v4", False):
        cfg.pop("v2", None)
        return build_nc_v4(ntiles, with_bias=with_bias, **cfg)
    if cfg.pop("v2", False):
        return build_nc_v2(ntiles, with_bias=with_bias, **cfg)
    return build_nc(ntiles, with_bias=with_bias, **cfg)


def get_nc(with_bias, cfg=None):
    cfg = dict(CFG if cfg is None else cfg)
    if with_bias and (cfg.get("v2") or cfg.get("v4")):
        cfg = dict(CFG_V1)  # v2/v4 assume zero bias
    key = (NTILES, with_bias, tuple(sorted(cfg.items())))
    if key not in _NC_CACHE:
        _NC_CACHE[key] = build_any(NTILES, with_bias=with_bias, cfg=cfg)
    return _NC_CACHE[key]


def make_in_maps(x, W, b, with_bias, cfg=None):
    cfg = dict(CFG if cfg is None else cfg)
    if with_bias and (cfg.get("v2") or cfg.get("v4")):
        cfg = dict(CFG_V1)
    if cfg.get("v2") or cfg.get("v4"):
        import ml_dtypes
        W = np.asarray(W, dtype=np.float32)
        wt = np.ascontiguousarray(
            W.transpose(0, 2, 1).reshape(DEPTH, 4, 128, SIZE)
            .transpose(2, 0, 1, 3))
        wt_hi = wt.astype(ml_dtypes.bfloat16)
        wt_lo = (wt - wt_hi.astype(np.float32)).astype(ml_dtypes.bfloat16)
        common = {"wt_hi": wt_hi, "wt_lo": wt_lo}
    else:
        common = _host_inputs(W, b, with_bias, cfg.get("t_on_pe", False),
                              cfg.get("mm_pair", False))
    shards = np.asarray(x, dtype=np.float32).reshape(-1, ROWS_PER_CORE, SIZE)
    return [dict(common, x=np.ascontiguousarray(shards[i]))
            for i in range(shards.shape[0])]


def kernel(x, W, b):
    from concourse import bass_utils

    x = np.asarray(x, dtype=np.float32)
    b = np.asarray(b, dtype=np.float32)
    assert x.shape == (BATCH, SIZE)
    with_bias = bool(np.any(b))
    nc = get_nc(with_bias)
    in_maps = make_in_maps(x, W, b, with_bias)
    res = bass_utils.run_bass_kernel_spmd(nc, in_maps,
                                          core_ids=list(range(N_CORES)))
    out = np.concatenate([res.results[i]["out"] for i in range(N_CORES)], axis=0)
    return out.astype(np.float32, copy=False)



# revision 26
# speedup vs baseline: 1.2606x; 1.2606x over previous
"""Trainium2 Bass kernel for nn_ACAClassifier (soft cellular-automaton update).

Reference computation, per depth d (8 depths):
    mask = sigmoid(state @ W[d].T + b[d])
    t    = 4*l + 2*c + r          (circular neighbors along feature axis)
    nb   = int(t)                 (truncation)
    bits = rule110_table[7 - nb]  == [t>=1] - [t>=4] + [t>=5] - [t>=7]
    state = mask*bits + (1-mask)*state

Sharding: pure data-parallel over the batch axis across 8 NeuronCores.

Default schedule (v4, tile-outer): chunks of CH row-tiles run through all
8 depths; NI chunks are interleaved in a flat software pipeline (feed
leads by feed_lead steps, blend trails by blend_lag) so the PE never
idles. W (bf16 hi/lo pair, host-pretransposed) stays SBUF-resident for
all depths. Per chunk-depth:
  - cast state to a bf16 pair (hi on Act, lo on Pool), XBAR-DMA-transpose
    both halves, 12 matmuls (3-term bf16-pair product, hi-terms first) to
    psum, sigmoid on Act.
  - t = (4l + r) + (2c - 4) via two fused affine_then_add DVE ops (the -4
    shift feeds the custom op's {-3,0,1,3} thresholds).
  - e = bits(t) - state in ONE custom DVE op (RULE_BITS_E_ANT, 8-stage
    is_ge chain, integer-edge exact); f = mask*e and state += f alternate
    between DVE and Pool; circular pad columns refresh on Pool.
v1 (build_nc) and v2 (build_nc_v2) depth-outer variants are kept for
reference/fallback (nonzero-bias inputs fall back to v1).
"""

import sys

for _p in ("/opt/pypackages", "/opt/trn_rl_repo"):
    if _p not in sys.path:
        sys.path.insert(0, _p)

import numpy as np

BATCH = 65536
SIZE = 512
DEPTH = 8
N_CORES = 8
ROWS_PER_CORE = BATCH // N_CORES  # 8192
NTILES = ROWS_PER_CORE // 128     # 64

_NC_CACHE = {}
_BITS_E_OP = None


def get_bits_e_op():
    """Custom fused DVE op: out = ([t>=1]-[t>=4]+[t>=5]-[t>=7]) - c.

    Exact rule-110 bits lookup (integer-edge exact) fused with the
    (bits - state) subtraction: one DVE pass instead of three.
    """
    global _BITS_E_OP
    if _BITS_E_OP is not None:
        return _BITS_E_OP
    from concourse.dve_spec import (AluOp, Bin, One, Spec, Src0, Src1, C0,
                                    C1, Zero)
    from concourse import dve_ops as DO

    # operates on u = t - 4 (the ata that builds t applies bias -4):
    # bits = [u>=-3] - [u>=0] + [u>=1] - [u>=3];  out = bits - c
    body = Bin(AluOp.SUBTRACT,
               Bin(AluOp.ADD,
                   Bin(AluOp.SUBTRACT,
                       Bin(AluOp.IS_GE, Src0, C0),
                       Bin(AluOp.IS_GE, Src0, Zero)),
                   Bin(AluOp.SUBTRACT,
                       Bin(AluOp.IS_GE, Src0, One),
                       Bin(AluOp.IS_GE, Src0, C1))),
               Src1)

    def ref(in0, in1, s0, s1, imm2):
        bits = ((in0 >= s0).astype(np.float32)
                - (in0 >= 0.0).astype(np.float32)
                + (in0 >= 1.0).astype(np.float32)
                - (in0 >= s1).astype(np.float32))
        return (bits - in1).astype(np.float32)

    spec = Spec(body=body, reference=ref)
    name = "RULE_BITS_E_ANT"
    if name not in DO._SUB_OPCODE_FOR_NAME:
        row = max(DO._SUB_OPCODE_FOR_NAME.values()) + 1
        assert row < 0x20, row
        DO._SUB_OPCODE_FOR_NAME[name] = row
    op = DO.DveOp(name, spec, subdim=False, uops_sha={})
    try:
        op.compile("v3")
    except ValueError as e:
        import re
        m = re.search(r"v3: ([0-9a-f]+)", str(e))
        if not m:
            raise
        op = DO.DveOp(name, spec, subdim=False, uops_sha={"v3": m.group(1)})
    op.compile("v3")
    if not any(o.name == name for o in DO.OPS):
        DO.OPS.append(op)
    _BITS_E_OP = op
    return op


def build_nc(ntiles, G=2, with_bias=False, t_on_pe=False, mm_pair=False,
             abs_on_sc=False, c2_on_sc=False, mm_f32r=False,
             num_devices=N_CORES, tmp_bufs=2, mask_bufs=3, st_bufs=4,
             psm_bufs=2, tb_bufs=None, repeat=1, skip_ew=False,
             skip_mm=False):
    """Build + compile the per-core Bass program (depth-outer schedule).

    mm_pair: 3-term bf16-pair mask matmul (s_hi@w_hi + s_hi@w_lo + s_lo@w_hi)
             instead of native fp32 (4 cyc/row -> 3x 1 cyc/row on PE).
    abs_on_sc: compute r1=|t-2.5|, r2=|t-6| on ScalarE; bits via is_lt on DVE.
    c2_on_sc: compute 2*c on ScalarE instead of DVE.
    """
    import concourse.bacc as bacc
    import concourse.mybir as mybir
    import concourse.tile as tile

    assert ntiles % G == 0
    ngroups = ntiles // G
    f32 = mybir.dt.float32
    f32r = mybir.dt.float32r
    bf16 = mybir.dt.bfloat16
    AL = mybir.AluOpType
    AF = mybir.ActivationFunctionType

    nc = bacc.Bacc("TRN2", target_bir_lowering=False, debug=False,
                   num_devices=num_devices)
    rows = ntiles * 128
    x_d = nc.dram_tensor("x", [rows, SIZE], f32, kind="ExternalInput")
    if mm_pair:
        wth_d = nc.dram_tensor("wt_hi", [128, DEPTH, 4, SIZE], bf16,
                               kind="ExternalInput")
        wtl_d = nc.dram_tensor("wt_lo", [128, DEPTH, 4, SIZE], bf16,
                               kind="ExternalInput")
    else:
        wt_d = nc.dram_tensor("wt", [128, DEPTH, 4, SIZE],
                              f32r if mm_f32r else f32,
                              kind="ExternalInput")
    id_d = nc.dram_tensor("ident", [128, 128], f32, kind="ExternalInput")
    if t_on_pe:
        ct_d = nc.dram_tensor("ct", [128, 130], f32, kind="ExternalInput")
    if with_bias:
        ones_d = nc.dram_tensor("ones", [1, 128], f32, kind="ExternalInput")
        b_d = nc.dram_tensor("b", [1, DEPTH * SIZE], f32, kind="ExternalInput")
    out_d = nc.dram_tensor("out", [rows, SIZE], f32, kind="ExternalOutput")

    with tile.TileContext(nc) as tc:
        with (
            tc.tile_pool(name="const", bufs=1) as constp,
            tc.tile_pool(name="state", bufs=1) as statep,
            tc.tile_pool(name="wtp", bufs=2) as wtp,
            tc.tile_pool(name="maskp", bufs=mask_bufs) as maskp,
            tc.tile_pool(name="tmpf", bufs=tmp_bufs) as tmpf,
            tc.tile_pool(name="tmpb", bufs=tmp_bufs) as tmpb,
            tc.tile_pool(name="stp", bufs=st_bufs) as stp,
            tc.tile_pool(name="psA", bufs=2, space="PSUM") as psA,
            tc.tile_pool(name="psM", bufs=psm_bufs, space="PSUM") as psM,
            tc.tile_pool(name="psT2", bufs=2, space="PSUM") as psT2,
        ):
            id_sb = constp.tile([128, 128], f32, tag="id")
            nc.sync.dma_start(id_sb[:], id_d.ap())
            if abs_on_sc:
                biasA = constp.tile([128, 1], f32, tag="biasA")
                nc.vector.memset(biasA[:], -2.5)
                biasB = constp.tile([128, 1], f32, tag="biasB")
                nc.vector.memset(biasB[:], -6.0)
            if t_on_pe:
                ct_sb = constp.tile([128, 130], f32, tag="ct")
                nc.sync.dma_start(ct_sb[:], ct_d.ap())
            if with_bias:
                ones_sb = constp.tile([1, 128], f32, tag="ones")
                nc.sync.dma_start(ones_sb[:], ones_d.ap())
                b_sb = constp.tile([1, DEPTH * SIZE], f32, tag="b")
                nc.sync.dma_start(b_sb[:], b_d.ap())

            x_ap = x_d.ap()
            wt_ap = None if mm_pair else wt_d.ap()
            out_ap = out_d.ap()

            sts = [statep.tile([128, G, SIZE + 2], f32, tag=f"st{g}",
                               name=f"st{g}")
                   for g in range(ngroups)]

            for rep in range(repeat):
                for g in range(ngroups):
                    st = sts[g]
                    for i in range(G):
                        r0 = (g * G + i) * 128
                        nc.sync.dma_start(st[:, i, 1:SIZE + 1],
                                          x_ap[r0:r0 + 128, :])
                    nc.vector.tensor_copy(st[:, :, 0:1], st[:, :, SIZE:SIZE + 1])
                    nc.vector.tensor_copy(st[:, :, SIZE + 1:SIZE + 2],
                                          st[:, :, 1:2])
                if skip_mm:
                    mask_const = maskp.tile([128, G, SIZE], f32, tag="maskc")
                    nc.vector.memset(mask_const[:], 0.5)

                for d in range(DEPTH):
                    if not skip_mm:
                        if mm_pair:
                            wth_sb = wtp.tile([128, 4, SIZE], bf16, tag="wth")
                            nc.sync.dma_start(wth_sb[:], wth_d.ap()[:, d, :, :])
                            wtl_sb = wtp.tile([128, 4, SIZE], bf16, tag="wtl")
                            nc.sync.dma_start(wtl_sb[:], wtl_d.ap()[:, d, :, :])
                        else:
                            wt_sb = wtp.tile([128, 4, SIZE],
                                             f32r if mm_f32r else f32,
                                             tag="wt")
                            nc.sync.dma_start(wt_sb[:], wt_ap[:, d, :, :])
                    for g in range(ngroups):
                        st = sts[g]
                        if skip_mm:
                            mask = mask_const
                        else:
                            pM = psM.tile([128, G, SIZE], f32, tag="pM")
                            mask = maskp.tile([128, G, SIZE], f32, tag="mask")
                        if t_on_pe:
                            g4 = tmpb.tile([128, G, SIZE], bf16, tag="g4")
                            g7 = tmpb.tile([128, G, SIZE], bf16, tag="g7")
                        for i in range(G if not skip_mm else 0):
                            pT = psA.tile([128, SIZE], f32, tag="pT")
                            for j in range(4):
                                nc.tensor.transpose(
                                    pT[:, j * 128:(j + 1) * 128],
                                    st[:, i, 1 + j * 128:1 + (j + 1) * 128],
                                    id_sb[:],
                                )
                            if mm_pair:
                                sTh = stp.tile([128, SIZE], bf16, tag="sTh")
                                nc.scalar.copy(sTh[:], pT[:])
                                sTl = stp.tile([128, SIZE], bf16, tag="sTl")
                                nc.vector.tensor_tensor(sTl[:], pT[:], sTh[:],
                                                        AL.subtract)
                                for j in range(4):
                                    cj = slice(j * 128, (j + 1) * 128)
                                    nc.tensor.matmul(
                                        pM[:, i, :], sTh[:, cj],
                                        wth_sb[:, j, :],
                                        start=(j == 0), stop=False)
                                    nc.tensor.matmul(
                                        pM[:, i, :], sTh[:, cj],
                                        wtl_sb[:, j, :],
                                        start=False, stop=False)
                                    nc.tensor.matmul(
                                        pM[:, i, :], sTl[:, cj],
                                        wth_sb[:, j, :],
                                        start=False,
                                        stop=(j == 3 and not with_bias))
                            else:
                                sT = stp.tile([128, SIZE],
                                              f32r if mm_f32r else f32,
                                              tag="sT")
                                nc.scalar.copy(sT[:], pT[:])
                                for j in range(4):
                                    nc.tensor.matmul(
                                        pM[:, i, :],
                                        sT[:, j * 128:(j + 1) * 128],
                                        wt_sb[:, j, :],
                                        start=(j == 0),
                                        stop=(j == 3 and not with_bias),
                                    )
                            if with_bias:
                                nc.tensor.matmul(
                                    pM[:, i, :],
                                    ones_sb[0:1, :],
                                    b_sb[0:1, d * SIZE:(d + 1) * SIZE],
                                    start=False,
                                    stop=True,
                                )
                            if t_on_pe:
                                # banded circulant t = 4l + 2c + r on PE:
                                # chunk j owns out cols [128j, 128j+127];
                                # cross-chunk neighbor contributions are
                                # added by 1-col accumulate fixups.
                                pT2 = psT2.tile([128, SIZE], f32, tag="pT2")
                                for j in range(4):
                                    nc.tensor.matmul(
                                        pT2[:, j * 128:(j + 1) * 128],
                                        sT[:, j * 128:(j + 1) * 128],
                                        ct_sb[:, 1:129],
                                        start=(j == 0), stop=False,
                                        skip_group_check=True)
                                for j in range(4):
                                    ca = (j * 128 + 128) % SIZE
                                    nc.tensor.matmul(
                                        pT2[:, ca:ca + 1],
                                        sT[:, j * 128:(j + 1) * 128],
                                        ct_sb[:, 129:130],
                                        start=False, stop=False,
                                        skip_group_check=True)
                                    cb = (j * 128 + SIZE - 1) % SIZE
                                    nc.tensor.matmul(
                                        pT2[:, cb:cb + 1],
                                        sT[:, j * 128:(j + 1) * 128],
                                        ct_sb[:, 0:1],
                                        start=False, stop=(j == 3),
                                        skip_group_check=True)
                                # per-tile compares straight from PSUM
                                nc.vector.tensor_scalar(
                                    g4[:, i, :], pT2[:], 4.0, None, AL.is_ge)
                                nc.vector.tensor_scalar(
                                    g7[:, i, :], pT2[:], 7.0, None, AL.is_ge)
                                nc.vector.scalar_tensor_tensor(
                                    g4[:, i, :], pT2[:], 1.0, g4[:, i, :],
                                    AL.is_ge, AL.subtract)
                                nc.vector.scalar_tensor_tensor(
                                    g7[:, i, :], pT2[:], 5.0, g7[:, i, :],
                                    AL.is_ge, AL.subtract)
                        if not skip_mm:
                            nc.scalar.activation(mask[:], pM[:], AF.Sigmoid)
                        if skip_ew:
                            continue

                        l_ap = st[:, :, 0:SIZE]
                        c_ap = st[:, :, 1:SIZE + 1]
                        r_ap = st[:, :, 2:SIZE + 2]
                        tb = tmpf.tile([128, G, SIZE], f32, tag="tb",
                                       bufs=tb_bufs)
                        if not t_on_pe:
                            c2 = tmpf.tile([128, G, SIZE], f32, tag="c2")
                            # u = 4*l + 2*c (both scalings exact, one add)
                            if c2_on_sc:
                                nc.scalar.mul(c2[:], c_ap, 2.0)
                            else:
                                nc.vector.tensor_scalar(c2[:], c_ap, 2.0, None,
                                                        AL.mult)
                            nc.vector.scalar_tensor_tensor(
                                c2[:], l_ap, 4.0, c2[:], AL.mult, AL.add)
                            # t = u + r
                            nc.vector.tensor_tensor(tb[:], c2[:], r_ap, AL.add)
                            t_src = tb[:]
                            g4 = tmpb.tile([128, G, SIZE], bf16, tag="g4")
                            g7 = tmpb.tile([128, G, SIZE], bf16, tag="g7")
                            if abs_on_sc:
                                # bits=1 iff t in [1,4) u [5,7):
                                # r1=|t-2.5|<1.5, r2=|t-6|<1 (exact-integer
                                # edge cases are measure-zero and accepted)
                                ra = tmpf.tile([128, G, SIZE], f32, tag="ra")
                                rb = tmpf.tile([128, G, SIZE], f32, tag="rb")
                                nc.scalar.activation(ra[:], t_src, AF.Abs,
                                                     bias=biasA[:])
                                nc.scalar.activation(rb[:], t_src, AF.Abs,
                                                     bias=biasB[:])
                                nc.vector.tensor_scalar(g4[:], ra[:], 1.5,
                                                        None, AL.is_lt)
                                nc.vector.tensor_scalar(g7[:], rb[:], 1.0,
                                                        None, AL.is_lt)
                            else:
                                nc.vector.tensor_scalar(g4[:], t_src, 4.0,
                                                        None, AL.is_ge)
                                nc.vector.tensor_scalar(g7[:], t_src, 7.0,
                                                        None, AL.is_ge)
                                # d1 = [t>=1]-[t>=4]; d2 = [t>=5]-[t>=7]
                                nc.vector.scalar_tensor_tensor(
                                    g4[:], t_src, 1.0, g4[:], AL.is_ge,
                                    AL.subtract)
                                nc.vector.scalar_tensor_tensor(
                                    g7[:], t_src, 5.0, g7[:], AL.is_ge,
                                    AL.subtract)
                        # bits = d1 + d2  (exact 0/1 in bf16)
                        nc.vector.tensor_tensor(g4[:], g4[:], g7[:], AL.add)
                        # e = bits - state
                        nc.vector.scalar_tensor_tensor(
                            tb[:], c_ap, -1.0, g4[:], AL.mult, AL.add)
                        # f = mask * e
                        nc.vector.tensor_tensor(tb[:], mask[:], tb[:], AL.mult)
                        # state += f   (in-place)
                        nc.vector.tensor_tensor(st[:, :, 1:SIZE + 1], tb[:],
                                                c_ap, AL.add)
                        # refresh circular-wrap pad columns
                        nc.vector.tensor_copy(st[:, :, 0:1],
                                              st[:, :, SIZE:SIZE + 1])
                        nc.vector.tensor_copy(st[:, :, SIZE + 1:SIZE + 2],
                                              st[:, :, 1:2])

                for g in range(ngroups):
                    st = sts[g]
                    for i in range(G):
                        r0 = (g * G + i) * 128
                        nc.sync.dma_start(out_ap[r0:r0 + 128, :],
                                          st[:, i, 1:SIZE + 1])

    nc.compile()
    return nc


def _host_inputs(W, b, with_bias, t_on_pe, mm_pair=False):
    import ml_dtypes

    W = np.asarray(W, dtype=np.float32)
    # wt[p, d, j, n] = W[d][n, j*128+p]
    wt = np.ascontiguousarray(
        W.transpose(0, 2, 1).reshape(DEPTH, 4, 128, SIZE).transpose(2, 0, 1, 3))
    common = {
        "ident": np.eye(128, dtype=np.float32),
    }
    if mm_pair:
        wt_hi = wt.astype(ml_dtypes.bfloat16)
        wt_lo = (wt - wt_hi.astype(np.float32)).astype(ml_dtypes.bfloat16)
        common["wt_hi"] = wt_hi
        common["wt_lo"] = wt_lo
    else:
        common["wt"] = wt
    if t_on_pe:
        # ct[k, m]: coeff of s_k in window col m: t_n = 4 s_{n-1} + 2 s_n
        # + s_{n+1}; main matmuls use ct[:,1:129], fixups cols 0 and 129.
        ct = np.zeros((128, 130), dtype=np.float32)
        for k in range(128):
            ct[k, k] = 1.0
            ct[k, k + 1] = 2.0
            ct[k, k + 2] = 4.0
        common["ct"] = ct
    if with_bias:
        common["ones"] = np.ones((1, 128), dtype=np.float32)
        common["b"] = np.ascontiguousarray(
            np.asarray(b, dtype=np.float32).reshape(1, DEPTH * SIZE))
    return common


def build_nc_v2(ntiles, G=4, with_bias=False, repeat=1, num_devices=N_CORES,
                wt_bufs=1, nat_bufs=1, st_bufs=2, mask_bufs=2, tmp_bufs=2,
                g_bufs=1, psm_bufs=2, r_halves=2, feed_span=1,
                eng_sh="scalar", eng_sl="gpsimd", eng_r="scalar",
                eng_g="vector", eng_bits="vector", eng_e="vector",
                eng_f="vector",
                eng_sp="gpsimd", use_ata=True, v2=True,
                skip_ew=False, skip_mm=False):
    """v2: no PE transposes (XBAR DMA transpose of the bf16 pair),
    3-term bf16-pair matmul, fused DVE ops, per-op engine assignment.

    Per depth, per group of G tiles (state resident as [128, G, 514] f32):
      split:  sh = bf16(state)         [eng_sh]
              sl = state - sh  (bf16)  [eng_sl]
      dmaT:   sTh, sTl = xbar-transpose(sh, sl)   [SP hwdge, 2 instrs]
      mm:     12 matmuls per tile -> pM psum      [PE]
      mask:   sigmoid(pM) -> sbuf                 [Act]
      t:      t = (4l + r) + 2c  (2x affine_then_add)   [DVE]
      bits:   r1=|t-2.5|, r2=|t-6| (fused ts absmax, 2x mode)
              g4=r1<1.5, g7=r2<1 (bf16), bits=g4+g7
      blend:  e = bits - c; f = mask*e; state += f  [DVE/gpsimd]
    """
    import concourse.bacc as bacc
    import concourse.mybir as mybir
    import concourse.tile as tile

    assert not with_bias, "v2 requires zero bias (falls back to v1)"
    assert ntiles % G == 0
    ngroups = ntiles // G
    f32 = mybir.dt.float32
    bf16 = mybir.dt.bfloat16
    AL = mybir.AluOpType
    AF = mybir.ActivationFunctionType

    nc = bacc.Bacc("TRN2", target_bir_lowering=False, debug=False,
                   num_devices=num_devices)
    rows = ntiles * 128
    x_d = nc.dram_tensor("x", [rows, SIZE], f32, kind="ExternalInput")
    wth_d = nc.dram_tensor("wt_hi", [128, DEPTH, 4, SIZE], bf16,
                           kind="ExternalInput")
    wtl_d = nc.dram_tensor("wt_lo", [128, DEPTH, 4, SIZE], bf16,
                           kind="ExternalInput")
    out_d = nc.dram_tensor("out", [rows, SIZE], f32, kind="ExternalOutput")

    def eng(name):
        return {"scalar": nc.scalar, "vector": nc.vector,
                "gpsimd": nc.gpsimd}[name]

    with tile.TileContext(nc) as tc:
        with (
            tc.tile_pool(name="const", bufs=1) as constp,
            tc.tile_pool(name="state", bufs=1) as statep,
            tc.tile_pool(name="wtp", bufs=wt_bufs) as wtp,
            tc.tile_pool(name="natp", bufs=nat_bufs) as natp,
            tc.tile_pool(name="stp", bufs=st_bufs) as stp,
            tc.tile_pool(name="maskp", bufs=mask_bufs) as maskp,
            tc.tile_pool(name="tmpp", bufs=tmp_bufs) as tmpp,
            tc.tile_pool(name="gp", bufs=g_bufs) as gp,
            tc.tile_pool(name="psM", bufs=psm_bufs, space="PSUM") as psM,
        ):
            x_ap = x_d.ap()
            out_ap = out_d.ap()
            if eng_r == "scalar":
                biasA = constp.tile([128, 1], f32, tag="biasA")
                nc.vector.memset(biasA[:], -2.5)
                biasB = constp.tile([128, 1], f32, tag="biasB")
                nc.vector.memset(biasB[:], -6.0)

            sts = [statep.tile([128, G, SIZE + 2], f32, tag=f"st{g}",
                               name=f"st{g}")
                   for g in range(ngroups)]

            for rep in range(repeat):
                for g in range(ngroups):
                    st = sts[g]
                    for i in range(G):
                        r0 = (g * G + i) * 128
                        nc.sync.dma_start(st[:, i, 1:SIZE + 1],
                                          x_ap[r0:r0 + 128, :])
                    nc.vector.tensor_copy(st[:, :, 0:1],
                                          st[:, :, SIZE:SIZE + 1])
                    nc.vector.tensor_copy(st[:, :, SIZE + 1:SIZE + 2],
                                          st[:, :, 1:2])
                if skip_mm:
                    mask_const = maskp.tile([128, G, SIZE], f32, tag="maskc")
                    nc.vector.memset(mask_const[:], 0.5)

                for d in range(DEPTH):
                    if not skip_mm:
                        wth_sb = wtp.tile([128, 4, SIZE], bf16, tag="wth")
                        nc.sync.dma_start(wth_sb[:], wth_d.ap()[:, d, :, :])
                        wtl_sb = wtp.tile([128, 4, SIZE], bf16, tag="wtl")
                        nc.sync.dma_start(wtl_sb[:], wtl_d.ap()[:, d, :, :])
                    for g in range(ngroups):
                        st = sts[g]
                        l_ap = st[:, :, 0:SIZE]
                        c_ap = st[:, :, 1:SIZE + 1]
                        r_ap = st[:, :, 2:SIZE + 2]

                        if skip_mm:
                            mask = mask_const
                        else:
                            # split state into exact bf16 pair, then XBAR
                            # transpose both halves; feeds batched over
                            # feed_span consecutive groups per DMA.
                            gi = g % feed_span
                            if gi == 0:
                                sh_sp = natp.tile([128, feed_span * G, SIZE],
                                                  bf16, tag="sh")
                                sl_sp = natp.tile([128, feed_span * G, SIZE],
                                                  bf16, tag="sl")
                                for g2 in range(g, g + feed_span):
                                    i2 = (g2 - g) * G
                                    c2_ap = sts[g2][:, :, 1:SIZE + 1]
                                    shs = sh_sp[:, i2:i2 + G, :]
                                    if eng_sh == "scalar":
                                        nc.scalar.copy(shs, c2_ap)
                                    else:
                                        eng(eng_sh).tensor_copy(shs, c2_ap)
                                    eng(eng_sl).tensor_tensor(
                                        sl_sp[:, i2:i2 + G, :], c2_ap, shs,
                                        AL.subtract)
                                sTh = stp.tile([128, feed_span * G * 4, 128],
                                               bf16, tag="sTh")
                                nc.sync.dma_start_transpose(sTh[:], sh_sp[:])
                                sTl = stp.tile([128, feed_span * G * 4, 128],
                                               bf16, tag="sTl")
                                nc.sync.dma_start_transpose(sTl[:], sl_sp[:])
                                span_sT = (sTh, sTl)
                            sTh, sTl = span_sT

                            pM = psM.tile([128, G, SIZE], f32, tag="pM")
                            for i in range(G):
                                for j in range(4):
                                    k = (gi * G + i) * 4 + j
                                    nc.tensor.matmul(
                                        pM[:, i, :], sTh[:, k, :],
                                        wth_sb[:, j, :],
                                        start=(j == 0), stop=False)
                                    nc.tensor.matmul(
                                        pM[:, i, :], sTh[:, k, :],
                                        wtl_sb[:, j, :],
                                        start=False, stop=False)
                                    nc.tensor.matmul(
                                        pM[:, i, :], sTl[:, k, :],
                                        wth_sb[:, j, :],
                                        start=False, stop=(j == 3))
                            mask = maskp.tile([128, G, SIZE], f32, tag="mask")
                            nc.scalar.activation(mask[:], pM[:], AF.Sigmoid)
                        if skip_ew:
                            continue

                        # t = 4l + 2c + r (exact fp32)
                        t = tmpp.tile([128, G, SIZE], f32, tag="tmp")
                        if use_ata:
                            nc.vector.affine_then_add(t[:], l_ap, r_ap,
                                                      4.0, 0.0)
                            nc.vector.affine_then_add(t[:], c_ap, t[:],
                                                      2.0, 0.0)
                        else:
                            nc.vector.tensor_scalar(t[:], l_ap, 4.0, None,
                                                    AL.mult)
                            nc.vector.scalar_tensor_tensor(
                                t[:], c_ap, 2.0, t[:], AL.mult, AL.add)
                            nc.vector.tensor_tensor(t[:], t[:], r_ap, AL.add)
                        # bits: r1=|t-2.5|<1.5 (covers [1,4)), r2=|t-6|<1
                        # (covers [5,7)); exact-integer edges measure-zero.
                        # r1 staged through a half-group scratch to save SBUF;
                        # r2 computed in place over t.
                        g4 = gp.tile([128, G, SIZE], bf16, tag="g4")
                        gh = G // r_halves
                        for h in range(r_halves):
                            hs = slice(h * gh, (h + 1) * gh)
                            r1 = tmpp.tile([128, gh, SIZE], f32, tag="rh",
                                           bufs=1)
                            if eng_r == "scalar":
                                nc.scalar.activation(r1[:], t[:, hs, :],
                                                     AF.Abs, bias=biasA[:])
                            else:
                                nc.vector.tensor_scalar(r1[:], t[:, hs, :],
                                                        2.5, None, AL.subtract)
                                nc.vector.tensor_scalar(r1[:], r1[:], 0.0,
                                                        None, AL.abs_max)
                            nc.vector.tensor_scalar(g4[:, hs, :], r1[:],
                                                    1.5, None, AL.is_lt)
                        if eng_r == "scalar":
                            nc.scalar.activation(t[:], t[:], AF.Abs,
                                                 bias=biasB[:])
                        else:
                            nc.vector.tensor_scalar(t[:], t[:], 6.0, None,
                                                    AL.subtract)
                            nc.vector.tensor_scalar(t[:], t[:], 0.0, None,
                                                    AL.abs_max)
                        g7 = gp.tile([128, G, SIZE], bf16, tag="g7")
                        eng(eng_g).tensor_scalar(g7[:], t[:], 1.0, None,
                                                 AL.is_lt)
                        eng(eng_bits).tensor_tensor(g4[:], g4[:], g7[:],
                                                    AL.add)
                        # blend: e = bits - c; f = mask*e; state += f
                        tb = tmpp.tile([128, G, SIZE], f32, tag="tmp")
                        eng(eng_e).scalar_tensor_tensor(
                            tb[:], c_ap, -1.0, g4[:], AL.mult, AL.add)
                        eng(eng_f).tensor_tensor(tb[:], mask[:], tb[:],
                                                 AL.mult)
                        eng(eng_sp).tensor_tensor(c_ap, tb[:], c_ap, AL.add)
                        # refresh circular-wrap pad columns
                        nc.vector.tensor_copy(st[:, :, 0:1],
                                              st[:, :, SIZE:SIZE + 1])
                        nc.vector.tensor_copy(st[:, :, SIZE + 1:SIZE + 2],
                                              st[:, :, 1:2])

                for g in range(ngroups):
                    st = sts[g]
                    for i in range(G):
                        r0 = (g * G + i) * 128
                        nc.sync.dma_start(out_ap[r0:r0 + 128, :],
                                          st[:, i, 1:SIZE + 1])

    nc.compile()
    return nc


def build_nc_v4(ntiles, CH=4, NI=2, repeat=1, num_devices=N_CORES,
                st_bufs=3, nat_bufs=2, stp_bufs=2, mask_bufs=2, t_bufs=2,
                g_bufs=2, e_bufs=2, psm_bufs=2,
                eng_sl="gpsimd", eng_t1="vector", eng_t2="vector",
                eng_g="vector", eng_b="vector", eng_e="vector",
                eng_f="vector", eng_u="vector", eng_pad="vector",
                feed_lead=2, blend_lag=1, trans_q="split", io_q="sp",
                stt_forms=False, bits_mod=False, with_bias=False,
                skip_ew=False, skip_mm=False):
    """v4: tile-outer schedule. Each chunk of CH row-tiles runs through all
    8 depths; NI chunks are interleaved so PE never idles. W (bf16 pair,
    host-pretransposed) stays SBUF-resident for all depths. State transposes
    via XBAR DMA of the bf16 split pair. Bits via the mod trick:
      bits' = [t>=7] - [(t mod 4)>=1]  ( == -bits )
      e' = c + bits';  f' = mask*e';  c -= f'
    Engine assignment per op is configurable; "alt" entries alternate
    vector/gpsimd per (chunk,depth) parity for fractional balance.
    """
    import concourse.bacc as bacc
    import concourse.mybir as mybir
    import concourse.tile as tile

    assert not with_bias, "v4 requires zero bias (falls back to v1)"
    assert ntiles % CH == 0
    nchunks = ntiles // CH
    f32 = mybir.dt.float32
    bf16 = mybir.dt.bfloat16
    AL = mybir.AluOpType
    AF = mybir.ActivationFunctionType

    bits_e_op = get_bits_e_op()
    nc = bacc.Bacc("TRN2", target_bir_lowering=False, debug=False,
                   num_devices=num_devices)
    rows = ntiles * 128
    x_d = nc.dram_tensor("x", [rows, SIZE], f32, kind="ExternalInput")
    wth_d = nc.dram_tensor("wt_hi", [128, DEPTH, 4, SIZE], bf16,
                           kind="ExternalInput")
    wtl_d = nc.dram_tensor("wt_lo", [128, DEPTH, 4, SIZE], bf16,
                           kind="ExternalInput")
    out_d = nc.dram_tensor("out", [rows, SIZE], f32, kind="ExternalOutput")

    def eng(name, k):
        if isinstance(name, (tuple, list)):
            name = name[k % len(name)]
        return {"vector": nc.vector, "gpsimd": nc.gpsimd,
                "scalar": nc.scalar}[name]

    def tt(e_, out, a, b_, op, k):
        # Pool only supports plain TensorTensor/TensorScalar/TensorCopy
        eng(e_, k).tensor_tensor(out, a, b_, op)

    with tile.TileContext(nc) as tc:
        with (
            tc.tile_pool(name="wres", bufs=1) as wres,
            tc.tile_pool(name="statep", bufs=st_bufs) as statep,
            tc.tile_pool(name="natp", bufs=nat_bufs) as natp,
            tc.tile_pool(name="stp", bufs=stp_bufs) as stp,
            tc.tile_pool(name="maskp", bufs=mask_bufs) as maskp,
            tc.tile_pool(name="tp", bufs=t_bufs) as tp,
            tc.tile_pool(name="gp", bufs=g_bufs) as gp,
            tc.tile_pool(name="ep", bufs=e_bufs) as ep,
            tc.tile_pool(name="psM", bufs=psm_bufs, space="PSUM") as psM,
        ):
            x_ap = x_d.ap()
            out_ap = out_d.ap()

            wth_sb = wres.tile([128, DEPTH, 4, SIZE], bf16, tag="wth")
            wtl_sb = wres.tile([128, DEPTH, 4, SIZE], bf16, tag="wtl")

            assert nchunks % NI == 0
            for rep in range(repeat):
                qio = {"sp": nc.sync, "act": nc.scalar,
                       "gpsimd": nc.gpsimd}[io_q]
                next_sts = {}

                def load_chunk(w, j):
                    ci = w * NI + j
                    st = statep.tile([128, CH, SIZE + 2], f32, tag="st")
                    for i in range(CH):
                        r0 = (ci * CH + i) * 128
                        qio.dma_start(st[:, i, 1:SIZE + 1],
                                      x_ap[r0:r0 + 128, :])
                    nc.vector.tensor_copy(st[:, :, 0:1],
                                          st[:, :, SIZE:SIZE + 1])
                    nc.vector.tensor_copy(st[:, :, SIZE + 1:SIZE + 2],
                                          st[:, :, 1:2])
                    return st

                for w in range(nchunks // NI):
                    if rep == 0 and w == 0:
                        # depth-0 weights first so the first matmuls are fed;
                        # later depths stream just-in-time during wave 0
                        nc.sync.dma_start(wth_sb[:, 0, :, :],
                                          wth_d.ap()[:, 0, :, :])
                        nc.sync.dma_start(wtl_sb[:, 0, :, :],
                                          wtl_d.ap()[:, 0, :, :])
                    sts = [next_sts.pop(j, None) for j in range(NI)]
                    for j in range(NI):
                        if sts[j] is None:
                            sts[j] = load_chunk(w, j)

                    # flat software pipeline over steps (d, j): the feed
                    # (cast + XBAR transposes) leads by feed_lead steps,
                    # crossing depth boundaries; the blend trails by one.
                    nsteps = DEPTH * NI
                    feeds = {}
                    pend = []

                    def emit_feed(srec):
                        d, j = srec
                        st = sts[j]
                        c_ap = st[:, :, 1:SIZE + 1]
                        k = (w * NI + j) * DEPTH + d
                        sh = natp.tile([128, CH, SIZE], bf16, tag="sh")
                        nc.scalar.copy(sh[:], c_ap)
                        sl = natp.tile([128, CH, SIZE], bf16, tag="sl")
                        tt(eng_sl, sl[:], c_ap, sh[:], AL.subtract, k)
                        qh = nc.scalar if trans_q in ("act", "split") \
                            else nc.sync
                        ql = nc.scalar if trans_q == "act" else nc.sync
                        sTh = stp.tile([128, CH * 4, 128], bf16, tag="sTh")
                        qh.dma_start_transpose(sTh[:], sh[:])
                        sTl = stp.tile([128, CH * 4, 128], bf16, tag="sTl")
                        ql.dma_start_transpose(sTl[:], sl[:])
                        feeds[(d, j)] = (sTh, sTl)

                    def emit_blend():
                        st, mask, e, k, d, j = pend.pop(0)
                        c_ap = st[:, :, 1:SIZE + 1]
                        tt(eng_f, e[:], mask[:], e[:], AL.mult, k)
                        tt(eng_u, c_ap, c_ap, e[:], AL.add, k)
                        if d == DEPTH - 1:
                            ci = w * NI + j
                            for i in range(CH):
                                r0 = (ci * CH + i) * 128
                                qio.dma_start(out_ap[r0:r0 + 128, :],
                                              st[:, i, 1:SIZE + 1])
                            if w + 1 < nchunks // NI:
                                # prefetch the next wave's chunk j now; the
                                # tile ring serializes on the out-store read
                                next_sts[j] = load_chunk(w + 1, j)

                    L = min(feed_lead, nsteps)
                    # blend(s) is emitted at the END of step s+blend_lag; the
                    # feed for step s+NI (same chunk, next depth) is emitted
                    # at the START of step s+NI-L and reads the state that
                    # blend(s)'s update writes -- so require
                    # blend_lag <= NI - L - 1.
                    assert blend_lag <= NI - L - 1, (NI, L, blend_lag)
                    for sidx in range(-L if not skip_mm else 0, nsteps):
                        if not skip_mm and sidx + L < nsteps:
                            emit_feed(divmod(sidx + L, NI))
                        if sidx < 0:
                            continue
                        d, j = divmod(sidx, NI)
                        if rep == 0 and w == 0 and j == 0 and d < DEPTH - 1:
                            nc.sync.dma_start(wth_sb[:, d + 1, :, :],
                                              wth_d.ap()[:, d + 1, :, :])
                            nc.sync.dma_start(wtl_sb[:, d + 1, :, :],
                                              wtl_d.ap()[:, d + 1, :, :])
                        ci = w * NI + j
                        st = sts[j]
                        k = ci * DEPTH + d
                        l_ap = st[:, :, 0:SIZE]
                        c_ap = st[:, :, 1:SIZE + 1]
                        r_ap = st[:, :, 2:SIZE + 2]

                        if not skip_mm:
                            sTh, sTl = feeds.pop((d, j))
                            pM = psM.tile([128, CH, SIZE], f32, tag="pM")
                            for i in range(CH):
                                # hi-pair matmuls first; sTl only needed for
                                # the trailing lo@hi group
                                for jj in range(4):
                                    kk = i * 4 + jj
                                    nc.tensor.matmul(
                                        pM[:, i, :], sTh[:, kk, :],
                                        wth_sb[:, d, jj, :],
                                        start=(jj == 0), stop=False,
                                        skip_group_check=True)
                                    nc.tensor.matmul(
                                        pM[:, i, :], sTh[:, kk, :],
                                        wtl_sb[:, d, jj, :],
                                        start=False, stop=False,
                                        skip_group_check=True)
                                for jj in range(4):
                                    kk = i * 4 + jj
                                    nc.tensor.matmul(
                                        pM[:, i, :], sTl[:, kk, :],
                                        wth_sb[:, d, jj, :],
                                        start=False, stop=(jj == 3),
                                        skip_group_check=True)
                            mask = maskp.tile([128, CH, SIZE], f32,
                                              tag="mask")
                            nc.scalar.activation(mask[:], pM[:], AF.Sigmoid)
                        else:
                            mask = maskp.tile([128, CH, SIZE], f32,
                                              tag="mask")
                            nc.vector.memset(mask[:], 0.5)
                        if skip_ew:
                            continue

                        # refresh circular pads from the previous depth's
                        # update (same dependency as t1: no extra blocking);
                        # the final depth needs no pads (output is 1..512)
                        if d > 0:
                            en = eng(eng_pad, k)
                            if en is nc.scalar:
                                nc.scalar.copy(st[:, :, 0:1],
                                               st[:, :, SIZE:SIZE + 1])
                                nc.scalar.copy(st[:, :, SIZE + 1:SIZE + 2],
                                               st[:, :, 1:2])
                            else:
                                en.tensor_copy(st[:, :, 0:1],
                                               st[:, :, SIZE:SIZE + 1])
                                en.tensor_copy(st[:, :, SIZE + 1:SIZE + 2],
                                               st[:, :, 1:2])
                        # t = (4l + r) + (2c - 4), exact fp32 (shifted by -4
                        # so the bits op thresholds are {-3, 0, 1, 3})
                        t = tp.tile([128, CH, SIZE], f32, tag="t")
                        eng(eng_t1, k).affine_then_add(t[:], l_ap, r_ap,
                                                       4.0, 0.0)
                        eng(eng_t2, k).affine_then_add(t[:], c_ap, t[:],
                                                       2.0, -4.0)
                        # e = bits(t) - c via the fused custom DVE op
                        e = ep.tile([128, CH, SIZE], f32, tag="e")
                        nc.vector._custom_dve(bits_e_op, out=e[:], in0=t[:],
                                              in1=c_ap, s0=-3.0, s1=3.0)
                        pend.append((st, mask, e, k, d, j))
                        if len(pend) > blend_lag:
                            emit_blend()
                    while pend:
                        emit_blend()

    nc.compile()
    return nc


# default configuration used by kernel(): depth-outer schedule, 3-term
# bf16-pair mask matmul, |t-c| bits layer on ScalarE, 2*c on ScalarE,
# triple-buffered mask psum.
CFG_V1 = dict(G=2, t_on_pe=False, mm_pair=True, abs_on_sc=True, c2_on_sc=True,
              psm_bufs=3)
# Pool-light schedule: real GPSIMD ops carry a per-op ucode launch cost
# the cost model underestimates, so keep Pool to ONE op per step (sl) and
# run f/u on DVE, pads on Act.
CFG = dict(v4=True, CH=4, NI=4, st_bufs=5, stp_bufs=3, nat_bufs=2,
           psm_bufs=2, t_bufs=1, e_bufs=3, mask_bufs=3, trans_q="sp",
           io_q="sp", eng_sl="gpsimd", eng_f="vector", eng_u="vector",
           eng_pad="scalar", blend_lag=1)


def build_any(ntiles, with_bias=False, cfg=None, **kw):
    cfg = dict(CFG if cfg is None else cfg)
    cfg.update(kw)
    if cfg.pop("v4", False):
        cfg.pop("v2", None)
        return build_nc_v4(ntiles, with_bias=with_bias, **cfg)
    if cfg.pop("v2", False):
        return build_nc_v2(ntiles, with_bias=with_bias, **cfg)
    return build_nc(ntiles, with_bias=with_bias, **cfg)


def get_nc(with_bias, cfg=None):
    cfg = dict(CFG if cfg is None else cfg)
    if with_bias and (cfg.get("v2") or cfg.get("v4")):
        cfg = dict(CFG_V1)  # v2/v4 assume zero bias
    key = (NTILES, with_bias, tuple(sorted(cfg.items())))
    if key not in _NC_CACHE:
        _NC_CACHE[key] = build_any(NTILES, with_bias=with_bias, cfg=cfg)
    return _NC_CACHE[key]


def make_in_maps(x, W, b, with_bias, cfg=None):
    cfg = dict(CFG if cfg is None else cfg)
    if with_bias and (cfg.get("v2") or cfg.get("v4")):
        cfg = dict(CFG_V1)
    if cfg.get("v2") or cfg.get("v4"):
        import ml_dtypes
        W = np.asarray(W, dtype=np.float32)
        wt = np.ascontiguousarray(
            W.transpose(0, 2, 1).reshape(DEPTH, 4, 128, SIZE)
            .transpose(2, 0, 1, 3))
        wt_hi = wt.astype(ml_dtypes.bfloat16)
        wt_lo = (wt - wt_hi.astype(np.float32)).astype(ml_dtypes.bfloat16)
        common = {"wt_hi": wt_hi, "wt_lo": wt_lo}
    else:
        common = _host_inputs(W, b, with_bias, cfg.get("t_on_pe", False),
                              cfg.get("mm_pair", False))
    shards = np.asarray(x, dtype=np.float32).reshape(-1, ROWS_PER_CORE, SIZE)
    return [dict(common, x=np.ascontiguousarray(shards[i]))
            for i in range(shards.shape[0])]


def kernel(x, W, b):
    from concourse import bass_utils

    x = np.asarray(x, dtype=np.float32)
    b = np.asarray(b, dtype=np.float32)
    assert x.shape == (BATCH, SIZE)
    with_bias = bool(np.any(b))
    nc = get_nc(with_bias)
    in_maps = make_in_maps(x, W, b, with_bias)
    res = bass_utils.run_bass_kernel_spmd(nc, in_maps,
                                          core_ids=list(range(N_CORES)))
    out = np.concatenate([res.results[i]["out"] for i in range(N_CORES)], axis=0)
    return out.astype(np.float32, copy=False)

